# revision 1
# baseline (speedup 1.0000x reference)
"""Trainium2 Bass kernel for nn_InteractionBlock (gnn_message_passing).

Key algebraic transformation: the reference computes, per angle alpha with
(s, t) = (src, tgt):
    sm[alpha] = (msg[s] @ Ws + bs) * d[t]
    out[alpha] = sum_b a[t, b] * (Wb[:, b, :] @ sm[alpha])
    agg[t] = sum_{alpha: tgt=t} out[alpha]
Everything except msg[s] depends only on t, so with
    S[t] = sum_{alpha: tgt=t} msg[s(alpha)]   and  c[t] = |{alpha: tgt=t}|
    agg[t] = sum_b a[t,b] * (Wb[:,b,:] @ ((S[t] @ Ws + c[t]*bs) * d[t]))
The A=600K gather+einsum+scatter collapses to an E-sized dense pipeline after
a segment-sum of raw msg rows -- done on device with indirect-DMA gathers and
TensorE transpose-accumulate. Edges are sharded across 8 cores; host does
index-only preprocessing (permutations / slot tables / layout transposes).
"""

import os
import sys

import numpy as np

sys.path.insert(0, "/opt/trn_rl_repo")

E = 100000
A = 600000
NR = 6
NS = 7
H = 128
BD = 8
M = 128
P = 8           # cores
ES = E // P     # 12500 edges per core
NT = 512        # dense-phase column tile
NCH = 100       # chunks of 128 targets per core (padded)
NSP = NCH * 128  # 12800 padded targets per core
NTILES = NSP // NT  # 25


def _preprocess(x_dist, x_angle, msg, angle_index):
    """Index-only + layout host prep. Returns per-core input maps and meta."""
    src = angle_index[0].astype(np.int64)
    tgt = angle_index[1].astype(np.int64)

    cnt = np.bincount(tgt, minlength=E).astype(np.int64)
    # sources grouped by target
    order = np.argsort(tgt, kind="stable")
    srcs_by_tgt = src[order].astype(np.int32)
    starts = np.zeros(E + 1, np.int64)
    np.cumsum(cnt, out=starts[1:])

    msg_pad = np.zeros((E + 1, M), np.float32)
    msg_pad[:E] = msg

    perms = []
    core_cols = []  # per core: ncols per chunk
    for p in range(P):
        lo = p * ES
        cp = cnt[lo:lo + ES]
        perm = np.argsort(-cp, kind="stable")
        perms.append(perm)
        cps = cp[perm]
        cps_pad = np.zeros(NSP, np.int64)
        cps_pad[:ES] = cps
        ncols = cps_pad.reshape(NCH, 128).max(axis=1)
        core_cols.append(ncols)
    ncols_common = np.maximum.reduce(core_cols)
    ncols_common = np.maximum(ncols_common, 1)  # every chunk gets >= 1 column
    tot_cols = int(ncols_common.sum())

    in_maps = []
    xa_flat = x_angle.reshape(E, NS * NR)
    for p in range(P):
        lo = p * ES
        perm = perms[p]
        gperm = perm + lo
        cps = cnt[gperm]

        # slot table: [128, tot_cols] int32; column r of chunk c, partition k
        # holds the j-th source of (new-label) target 128c+k, or E (zero row).
        slot = np.full((NSP, int(ncols_common.max())), E, np.int32)
        # fill rows: for local new-label i (i < ES), sources of gperm[i]
        reps = cps  # how many slots filled per row
        ii = np.repeat(np.arange(ES), reps)
        jj = (np.arange(len(ii)) -
              np.repeat(np.concatenate(([0], np.cumsum(reps)[:-1])), reps))
        gt = np.repeat(gperm, reps)
        # source list offsets
        sstart = starts[gt] + jj
        slot[ii, jj] = srcs_by_tgt[sstart]

        cols = np.concatenate(
            [slot[c * 128:(c + 1) * 128, :ncols_common[c]] for c in range(NCH)],
            axis=1,
        )  # [128, tot_cols]
        assert cols.shape == (128, tot_cols)

        pad = NSP - ES
        xdT = np.zeros((NR, NSP), np.float32)
        xdT[:, :ES] = x_dist[lo:lo + ES][perm].T
        xaT = np.zeros((NS * NR, NSP), np.float32)
        xaT[:, :ES] = xa_flat[lo:lo + ES][perm].T
        msgT = np.zeros((M, NSP), np.float32)
        msgT[:, :ES] = msg[lo:lo + ES][perm].T
        counts_row = np.zeros((1, NSP), np.float32)
        counts_row[0, :ES] = cps.astype(np.float32)

        in_maps.append({
            "msg_pad": msg_pad,
            "slots": np.ascontiguousarray(cols),
            "xdT": xdT,
            "xaT": xaT,
            "msgT": msgT,
            "counts": counts_row,
        })
    return in_maps, perms, ncols_common


def _weights_maps(Wd, Wa, Ws, bs, Wt, bt, Wb, rb_w1, rb_b1, rb_w2, rb_b2,
                  Wskip, bskip, ra1_w1, ra1_b1, ra1_w2, ra1_b2,
                  ra2_w1, ra2_b1, ra2_w2, ra2_b2):
    f32 = np.float32
    WaRep = np.empty((NS * NR, BD * 128), f32)
    for b in range(BD):
        WaRep[:, b * 128:(b + 1) * 128] = Wa[:, b:b + 1]  # replicate col b
    WbT = np.empty((H, BD * H), f32)
    for b in range(BD):
        WbT[:, b * 128:(b + 1) * 128] = Wb[:, b, :].T  # [h, i]
    return {
        "Wd": Wd.astype(f32), "WaRep": WaRep,
        "Ws": Ws.astype(f32), "bs_row": bs.reshape(1, H).astype(f32),
        "Wt": Wt.astype(f32), "bt_row": bt.reshape(1, H).astype(f32),
        "WbT": WbT,
        "rb_w1": rb_w1.astype(f32), "rb_b1": rb_b1.reshape(H, 1).astype(f32),
        "rb_w2": rb_w2.astype(f32), "rb_b2": rb_b2.reshape(H, 1).astype(f32),
        "Wskip": Wskip.astype(f32), "bskip": bskip.reshape(M, 1).astype(f32),
        "ra1_w1": ra1_w1.astype(f32), "ra1_b1": ra1_b1.reshape(M, 1).astype(f32),
        "ra1_w2": ra1_w2.astype(f32), "ra1_b2": ra1_b2.reshape(M, 1).astype(f32),
        "ra2_w1": ra2_w1.astype(f32), "ra2_b1": ra2_b1.reshape(M, 1).astype(f32),
        "ra2_w2": ra2_w2.astype(f32), "ra2_b2": ra2_b2.reshape(M, 1).astype(f32),
    }


def _build(nc, tc, ncols, aps, mode="full", reps=1):
    """Emit the kernel IR. aps: dict name -> bass AP of DRAM tensors.
    mode: "full" | "phase1" (segment-sum only) | "dense" (skip gathers).
    reps: repeat the computation body (for ablation timing via deltas)."""
    from contextlib import ExitStack

    import concourse.bass as bass
    from concourse import mybir
    from concourse.bass import IndirectOffsetOnAxis
    from concourse.masks import make_identity

    f32 = mybir.dt.float32
    Silu = mybir.ActivationFunctionType.Silu
    mult = mybir.AluOpType.mult
    tot_cols = int(ncols.sum())

    with ExitStack() as ctx:
        wpool = ctx.enter_context(tc.tile_pool(name="w", bufs=1))
        stpool = ctx.enter_context(tc.tile_pool(name="st", bufs=1))

        # resident weights
        wt = {}
        for name, shape in [
            ("Wd", (NR, H)), ("WaRep", (NS * NR, BD * 128)),
            ("Ws", (M, H)), ("bs_row", (1, H)),
            ("Wt", (M, H)), ("bt_row", (1, H)), ("WbT", (H, BD * H)),
            ("rb_w1", (H, H)), ("rb_b1", (H, 1)),
            ("rb_w2", (H, H)), ("rb_b2", (H, 1)),
            ("Wskip", (H, M)), ("bskip", (M, 1)),
            ("ra1_w1", (M, M)), ("ra1_b1", (M, 1)),
            ("ra1_w2", (M, M)), ("ra1_b2", (M, 1)),
            ("ra2_w1", (M, M)), ("ra2_b1", (M, 1)),
            ("ra2_w2", (M, M)), ("ra2_b2", (M, 1)),
        ]:
            t = wpool.tile(list(shape), f32, tag=name)
            nc.sync.dma_start(t[:], aps[name][:])
            wt[name] = t

        ident = wpool.tile([128, 128], f32, tag="ident")
        make_identity(nc, ident[:])
        ones_row = wpool.tile([1, NSP], f32, tag="ones")
        nc.gpsimd.memset(ones_row[:], 1.0)

        slots_sb = wpool.tile([128, tot_cols], mybir.dt.int32, tag="slots")
        nc.sync.dma_start(slots_sb[:], aps["slots"][:])

        ST = stpool.tile([M, NSP], f32, tag="ST")  # feature-major segment sums

        # ---- phase 1: S^T[:, 128c:128c+128] = sum_j gather_j^T --------------
        if mode == "dense":
            nc.gpsimd.memset(ST[:], 0.0)
        else:
            with tc.tile_pool(name="g", bufs=12) as gpool, \
                 tc.tile_pool(name="pt", bufs=6, space="PSUM") as ptpool:
                col = 0
                for c in range(NCH):
                    nj = int(ncols[c])
                    ps = ptpool.tile([128, 128], f32, tag="pt")
                    for j in range(nj):
                        g = gpool.tile([128, M], f32, tag="g")
                        nc.gpsimd.indirect_dma_start(
                            out=g[:], out_offset=None,
                            in_=aps["msg_pad"][:],
                            in_offset=IndirectOffsetOnAxis(
                                ap=slots_sb[:, col + j:col + j + 1], axis=0),
                        )
                        nc.tensor.matmul(
                            ps[:], lhsT=g[:], rhs=ident[:], is_transpose=True,
                            start=(j == 0), stop=(j == nj - 1),
                            skip_group_check=True,
                        )
                    nc.scalar.copy(ST[:, c * 128:(c + 1) * 128], ps[:])
                    col += nj

        if mode == "phase1":
            nc.sync.dma_start(aps["outT"][:], ST[:])
            return

        # ---- phase 2: dense per-edge pipeline, feature-major ----------------
        dense = ctx.enter_context(tc.tile_pool(name="dn", bufs=3))
        pacc = ctx.enter_context(tc.tile_pool(name="pacc", bufs=2, space="PSUM"))
        psc = ctx.enter_context(tc.tile_pool(name="psc", bufs=4, space="PSUM"))

        def mm(out, lhsT, rhs, start=True, stop=True):
            nc.tensor.matmul(out[:], lhsT=lhsT[:], rhs=rhs[:], start=start,
                             stop=stop, skip_group_check=True)

        for t in range(NTILES):
            sl = slice(t * NT, (t + 1) * NT)

            msgT_t = dense.tile([M, NT], f32, tag="msgT")
            nc.sync.dma_start(msgT_t[:], aps["msgT"][:, sl])
            xdT_t = dense.tile([NR, NT], f32, tag="xdT")
            nc.sync.dma_start(xdT_t[:], aps["xdT"][:, sl])
            xaT_t = dense.tile([NS * NR, NT], f32, tag="xaT")
            nc.sync.dma_start(xaT_t[:], aps["xaT"][:, sl])
            cnt_t = dense.tile([1, NT], f32, tag="cnt")
            nc.sync.dma_start(cnt_t[:], aps["counts"][:, sl])

            # d = x_dist @ Wd
            ps_d = psc.tile([H, NT], f32, tag="ps")
            mm(ps_d, wt["Wd"], xdT_t)
            d_sb = dense.tile([H, NT], f32, tag="d")
            nc.scalar.copy(d_sb[:], ps_d[:])

            # u = (S@Ws + c*bs) * d
            ps_u = psc.tile([H, NT], f32, tag="ps")
            nc.tensor.matmul(ps_u[:], lhsT=wt["Ws"][:], rhs=ST[:, sl],
                             start=True, stop=False, skip_group_check=True)
            mm(ps_u, wt["bs_row"], cnt_t, start=False, stop=True)
            u_sb = dense.tile([H, NT], f32, tag="u")
            nc.vector.tensor_tensor(out=u_sb[:], in0=ps_u[:], in1=d_sb[:], op=mult)

            # x0 = agg + msg@Wt + bt    (accumulated in one PSUM tile)
            ps_x0 = pacc.tile([H, NT], f32, tag="pacc")
            mm(ps_x0, wt["Wt"], msgT_t, start=True, stop=False)
            mm(ps_x0, wt["bt_row"], ones_row[:, sl], start=False, stop=False)
            for b in range(BD):
                bsl = slice(b * 128, (b + 1) * 128)
                ps_a = psc.tile([H, NT], f32, tag="ps")
                mm(ps_a, wt["WaRep"][:, bsl], xaT_t)
                z_sb = dense.tile([H, NT], f32, tag="z")
                nc.vector.tensor_tensor(out=z_sb[:], in0=u_sb[:], in1=ps_a[:],
                                        op=mult)
                mm(ps_x0, wt["WbT"][:, bsl], z_sb, start=False,
                   stop=(b == BD - 1))
            x0_sb = dense.tile([H, NT], f32, tag="x0")
            nc.scalar.copy(x0_sb[:], ps_x0[:])

            # residual block (H)
            ps_h = psc.tile([H, NT], f32, tag="ps")
            mm(ps_h, wt["rb_w1"], x0_sb)
            h1_sb = dense.tile([H, NT], f32, tag="h1")
            nc.scalar.activation(h1_sb[:], ps_h[:], Silu, bias=wt["rb_b1"][:])
            ps_h2 = psc.tile([H, NT], f32, tag="ps")
            mm(ps_h2, wt["rb_w2"], h1_sb)
            h2_sb = dense.tile([H, NT], f32, tag="h2")
            nc.scalar.activation(h2_sb[:], ps_h2[:], Silu, bias=wt["rb_b2"][:])

            # skip: y = silu((x0+h2)@Wskip + bskip) + msg
            ps_y = pacc.tile([H, NT], f32, tag="pacc")
            mm(ps_y, wt["Wskip"], x0_sb, start=True, stop=False)
            mm(ps_y, wt["Wskip"], h2_sb, start=False, stop=True)
            ys_sb = dense.tile([M, NT], f32, tag="ys")
            nc.scalar.activation(ys_sb[:], ps_y[:], Silu, bias=wt["bskip"][:])
            y_sb = dense.tile([M, NT], f32, tag="y")
            nc.vector.tensor_add(out=y_sb[:], in0=ys_sb[:], in1=msgT_t[:])

            # residual after 1
            ps_h = psc.tile([M, NT], f32, tag="ps")
            mm(ps_h, wt["ra1_w1"], y_sb)
            h1p = dense.tile([M, NT], f32, tag="h1")
            nc.scalar.activation(h1p[:], ps_h[:], Silu, bias=wt["ra1_b1"][:])
            ps_h2 = psc.tile([M, NT], f32, tag="ps")
            mm(ps_h2, wt["ra1_w2"], h1p)
            h2p = dense.tile([M, NT], f32, tag="h2")
            nc.scalar.activation(h2p[:], ps_h2[:], Silu, bias=wt["ra1_b2"][:])
            x2_sb = dense.tile([M, NT], f32, tag="x2")
            nc.vector.tensor_add(out=x2_sb[:], in0=y_sb[:], in1=h2p[:])

            # residual after 2
            ps_h = psc.tile([M, NT], f32, tag="ps")
            mm(ps_h, wt["ra2_w1"], x2_sb)
            h1q = dense.tile([M, NT], f32, tag="h1")
            nc.scalar.activation(h1q[:], ps_h[:], Silu, bias=wt["ra2_b1"][:])
            ps_h2 = psc.tile([M, NT], f32, tag="ps")
            mm(ps_h2, wt["ra2_w2"], h1q)
            h2q = dense.tile([M, NT], f32, tag="h2")
            nc.scalar.activation(h2q[:], ps_h2[:], Silu, bias=wt["ra2_b2"][:])
            o_sb = dense.tile([M, NT], f32, tag="o")
            nc.vector.tensor_add(out=o_sb[:], in0=x2_sb[:], in1=h2q[:])

            nc.sync.dma_start(aps["outT"][:, sl], o_sb[:])


def kernel(**inputs):
    inputs = {k: np.asarray(v) for k, v in inputs.items()}
    x_dist = inputs["x_dist"].astype(np.float32)
    x_angle = inputs["x_angle"].astype(np.float32)
    msg = inputs["msg"].astype(np.float32)
    angle_index = inputs["angle_index"]

    in_maps, perms, ncols = _preprocess(x_dist, x_angle, msg, angle_index)
    wmap = _weights_maps(**{k: inputs[k] for k in (
        "Wd", "Wa", "Ws", "bs", "Wt", "bt", "Wb",
        "rb_w1", "rb_b1", "rb_w2", "rb_b2", "Wskip", "bskip",
        "ra1_w1", "ra1_b1", "ra1_w2", "ra1_b2",
        "ra2_w1", "ra2_b1", "ra2_w2", "ra2_b2")})
    for im in in_maps:
        im.update(wmap)

    import concourse.bass as bass
    import concourse.tile as tile
    from concourse import bacc, mybir
    from concourse import bass_utils

    nc = bacc.Bacc("TRN2", target_bir_lowering=False, debug=False,
                   enable_asserts=False, num_devices=P)
    aps = {}
    for name, arr in in_maps[0].items():
        aps[name] = nc.dram_tensor(
            name, arr.shape, mybir.dt.from_np(arr.dtype),
            kind="ExternalInput").ap()
    aps["outT"] = nc.dram_tensor(
        "outT", (M, NSP), mybir.dt.float32, kind="ExternalOutput").ap()

    with tile.TileContext(nc) as tc:
        _build(nc, tc, ncols, aps)
    nc.compile()

    res = bass_utils.run_bass_kernel_spmd(
        nc, in_maps, core_ids=list(range(P)),
        trace=bool(int(os.environ.get("KTRACE", "0"))),
        stitch_traces=bool(int(os.environ.get("KTRACE", "0"))),
        trace_cores=list(range(P)) if int(os.environ.get("KTRACE", "0")) else None,
    )
    kernel.last_results = res

    out = np.empty((E, M), np.float32)
    for p in range(P):
        outT = res.results[p]["outT"]  # [M, NSP]
        out[p * ES + perms[p]] = outT[:, :ES].T
    return out.astype(inputs["msg"].dtype if inputs["msg"].dtype == np.float32
                      else np.float32)



# revision 3
# speedup vs baseline: 1.3008x; 1.3008x over previous
"""Trainium2 Bass kernel for nn_InteractionBlock (gnn_message_passing).

Key algebraic transformation: the reference computes, per angle alpha with
(s, t) = (src, tgt):
    sm[alpha] = (msg[s] @ Ws + bs) * d[t]
    out[alpha] = sum_b a[t, b] * (Wb[:, b, :] @ sm[alpha])
    agg[t] = sum_{alpha: tgt=t} out[alpha]
Everything except msg[s] depends only on t, so with
    S[t] = sum_{alpha: tgt=t} msg[s(alpha)]   and  c[t] = |{alpha: tgt=t}|
    agg[t] = sum_b a[t,b] * (Wb[:,b,:] @ ((S[t] @ Ws + c[t]*bs) * d[t]))
The A=600K gather+einsum+scatter collapses to an E-sized dense pipeline after
a segment-sum of raw msg rows -- done on device with indirect-DMA gathers and
TensorE transpose-accumulate. Edges are sharded across 8 cores; host does
index-only preprocessing (permutations / slot tables / layout transposes).
"""

import os
import sys
import time

import numpy as np

sys.path.insert(0, "/opt/trn_rl_repo")

_T0 = time.perf_counter()


def _tlog(msg):
    print(f"[ktime +{time.perf_counter() - _T0:7.2f}s] {msg}", file=sys.stderr)

E = 100000
A = 600000
NR = 6
NS = 7
H = 128
BD = 8
M = 128
P = 8           # cores
ES = E // P     # 12500 edges per core
NT = 512        # dense-phase column tile
NCH = 100       # chunks of 128 targets per core (padded)
NSP = NCH * 128  # 12800 padded targets per core
NTILES = NSP // NT  # 25


def _preprocess(x_dist, x_angle, msg, angle_index):
    """Index-only + layout host prep. Returns per-core input maps and meta."""
    src = angle_index[0].astype(np.int64)
    tgt = angle_index[1].astype(np.int64)

    cnt = np.bincount(tgt, minlength=E).astype(np.int64)
    # sources grouped by target
    order = np.argsort(tgt, kind="stable")
    srcs_by_tgt = src[order].astype(np.int32)
    starts = np.zeros(E + 1, np.int64)
    np.cumsum(cnt, out=starts[1:])

    msg_pad = np.zeros((E + 1, M), np.float32)
    msg_pad[:E] = msg

    perms = []
    core_cols = []  # per core: ncols per chunk
    for p in range(P):
        lo = p * ES
        cp = cnt[lo:lo + ES]
        perm = np.argsort(-cp, kind="stable")
        perms.append(perm)
        cps = cp[perm]
        cps_pad = np.zeros(NSP, np.int64)
        cps_pad[:ES] = cps
        ncols = cps_pad.reshape(NCH, 128).max(axis=1)
        core_cols.append(ncols)
    ncols_common = np.maximum.reduce(core_cols)
    ncols_common = np.maximum(ncols_common, 1)  # every chunk gets >= 1 column
    tot_cols = int(ncols_common.sum())

    in_maps = []
    xa_flat = x_angle.reshape(E, NS * NR)
    for p in range(P):
        lo = p * ES
        perm = perms[p]
        gperm = perm + lo
        cps = cnt[gperm]

        # slot table: [128, tot_cols] int32; column r of chunk c, partition k
        # holds the j-th source of (new-label) target 128c+k, or E (zero row).
        slot = np.full((NSP, int(ncols_common.max())), E, np.int32)
        # fill rows: for local new-label i (i < ES), sources of gperm[i]
        reps = cps  # how many slots filled per row
        ii = np.repeat(np.arange(ES), reps)
        jj = (np.arange(len(ii)) -
              np.repeat(np.concatenate(([0], np.cumsum(reps)[:-1])), reps))
        gt = np.repeat(gperm, reps)
        # source list offsets
        sstart = starts[gt] + jj
        slot[ii, jj] = srcs_by_tgt[sstart]

        cols = np.concatenate(
            [slot[c * 128:(c + 1) * 128, :ncols_common[c]] for c in range(NCH)],
            axis=1,
        )  # [128, tot_cols]
        assert cols.shape == (128, tot_cols)

        pad = NSP - ES
        xdT = np.zeros((NR, NSP), np.float32)
        xdT[:, :ES] = x_dist[lo:lo + ES][perm].T
        xaT = np.zeros((NS * NR, NSP), np.float32)
        xaT[:, :ES] = xa_flat[lo:lo + ES][perm].T
        msgT = np.zeros((M, NSP), np.float32)
        msgT[:, :ES] = msg[lo:lo + ES][perm].T
        counts_row = np.zeros((1, NSP), np.float32)
        counts_row[0, :ES] = cps.astype(np.float32)

        in_maps.append({
            "msg_pad": msg_pad,
            "slots": np.ascontiguousarray(cols),
            "xdT": xdT,
            "xaT": xaT,
            "msgT": msgT,
            "counts": counts_row,
        })
    return in_maps, perms, ncols_common


def _weights_maps(Wd, Wa, Ws, bs, Wt, bt, Wb, rb_w1, rb_b1, rb_w2, rb_b2,
                  Wskip, bskip, ra1_w1, ra1_b1, ra1_w2, ra1_b2,
                  ra2_w1, ra2_b1, ra2_w2, ra2_b2):
    f32 = np.float32
    WaRep = np.empty((NS * NR, BD * 128), f32)
    for b in range(BD):
        WaRep[:, b * 128:(b + 1) * 128] = Wa[:, b:b + 1]  # replicate col b
    WbT = np.empty((H, BD * H), f32)
    for b in range(BD):
        WbT[:, b * 128:(b + 1) * 128] = Wb[:, b, :].T  # [h, i]
    return {
        "Wd": Wd.astype(f32), "WaRep": WaRep,
        "Ws": Ws.astype(f32), "bs_row": bs.reshape(1, H).astype(f32),
        "Wt": Wt.astype(f32), "bt_row": bt.reshape(1, H).astype(f32),
        "WbT": WbT,
        "rb_w1": rb_w1.astype(f32), "rb_b1": rb_b1.reshape(H, 1).astype(f32),
        "rb_w2": rb_w2.astype(f32), "rb_b2": rb_b2.reshape(H, 1).astype(f32),
        "Wskip": Wskip.astype(f32), "bskip": bskip.reshape(M, 1).astype(f32),
        "ra1_w1": ra1_w1.astype(f32), "ra1_b1": ra1_b1.reshape(M, 1).astype(f32),
        "ra1_w2": ra1_w2.astype(f32), "ra1_b2": ra1_b2.reshape(M, 1).astype(f32),
        "ra2_w1": ra2_w1.astype(f32), "ra2_b1": ra2_b1.reshape(M, 1).astype(f32),
        "ra2_w2": ra2_w2.astype(f32), "ra2_b2": ra2_b2.reshape(M, 1).astype(f32),
    }


def _build(nc, tc, ncols, aps, mode="full", reps=1):
    """Emit the kernel IR. aps: dict name -> bass AP of DRAM tensors.
    mode: "full" | "phase1" (segment-sum only) | "dense" (skip gathers).
    reps: repeat the computation body (for ablation timing via deltas)."""
    from contextlib import ExitStack

    import concourse.bass as bass
    from concourse import mybir
    from concourse.bass import IndirectOffsetOnAxis
    from concourse.masks import make_identity

    f32 = mybir.dt.float32
    Silu = mybir.ActivationFunctionType.Silu
    mult = mybir.AluOpType.mult
    tot_cols = int(ncols.sum())

    with ExitStack() as ctx:
        wpool = ctx.enter_context(tc.tile_pool(name="w", bufs=1))
        stpool = ctx.enter_context(tc.tile_pool(name="st", bufs=1))

        # resident weights
        wt = {}
        for name, shape in [
            ("Wd", (NR, H)), ("WaRep", (NS * NR, BD * 128)),
            ("Ws", (M, H)), ("bs_row", (1, H)),
            ("Wt", (M, H)), ("bt_row", (1, H)), ("WbT", (H, BD * H)),
            ("rb_w1", (H, H)), ("rb_b1", (H, 1)),
            ("rb_w2", (H, H)), ("rb_b2", (H, 1)),
            ("Wskip", (H, M)), ("bskip", (M, 1)),
            ("ra1_w1", (M, M)), ("ra1_b1", (M, 1)),
            ("ra1_w2", (M, M)), ("ra1_b2", (M, 1)),
            ("ra2_w1", (M, M)), ("ra2_b1", (M, 1)),
            ("ra2_w2", (M, M)), ("ra2_b2", (M, 1)),
        ]:
            t = wpool.tile(list(shape), f32, tag=name)
            nc.sync.dma_start(t[:], aps[name][:])
            wt[name] = t

        ident = wpool.tile([128, 128], f32, tag="ident")
        make_identity(nc, ident[:])
        ones_row = wpool.tile([1, NSP], f32, tag="ones")
        nc.gpsimd.memset(ones_row[:], 1.0)

        slots_sb = wpool.tile([128, tot_cols], mybir.dt.int32, tag="slots")
        nc.sync.dma_start(slots_sb[:], aps["slots"][:])

        ST = stpool.tile([M, NSP], f32, tag="ST")  # feature-major segment sums

        # ---- phase 1: S^T[:, 128c:128c+128] = sum_j gather_j^T --------------
        if mode == "dense":
            nc.gpsimd.memset(ST[:], 0.0)
        else:
            with tc.tile_pool(name="g", bufs=12) as gpool, \
                 tc.tile_pool(name="pt", bufs=6, space="PSUM") as ptpool:
                col = 0
                for c in range(NCH):
                    nj = int(ncols[c])
                    ps = ptpool.tile([128, 128], f32, tag="pt")
                    for j in range(nj):
                        g = gpool.tile([128, M], f32, tag="g")
                        nc.gpsimd.indirect_dma_start(
                            out=g[:], out_offset=None,
                            in_=aps["msg_pad"][:],
                            in_offset=IndirectOffsetOnAxis(
                                ap=slots_sb[:, col + j:col + j + 1], axis=0),
                        )
                        nc.tensor.matmul(
                            ps[:], lhsT=g[:], rhs=ident[:], is_transpose=True,
                            start=(j == 0), stop=(j == nj - 1),
                            skip_group_check=True,
                        )
                    nc.scalar.copy(ST[:, c * 128:(c + 1) * 128], ps[:])
                    col += nj

        if mode == "phase1":
            nc.sync.dma_start(aps["outT"][:], ST[:])
            return

        # ---- phase 2: dense per-edge pipeline, feature-major ----------------
        dense = ctx.enter_context(tc.tile_pool(name="dn", bufs=3))
        pacc = ctx.enter_context(tc.tile_pool(name="pacc", bufs=2, space="PSUM"))
        psc = ctx.enter_context(tc.tile_pool(name="psc", bufs=4, space="PSUM"))

        def mm(out, lhsT, rhs, start=True, stop=True):
            nc.tensor.matmul(out[:], lhsT=lhsT[:], rhs=rhs[:], start=start,
                             stop=stop, skip_group_check=True)

        for t in range(NTILES):
            sl = slice(t * NT, (t + 1) * NT)

            msgT_t = dense.tile([M, NT], f32, tag="msgT")
            nc.sync.dma_start(msgT_t[:], aps["msgT"][:, sl])
            xdT_t = dense.tile([NR, NT], f32, tag="xdT")
            nc.sync.dma_start(xdT_t[:], aps["xdT"][:, sl])
            xaT_t = dense.tile([NS * NR, NT], f32, tag="xaT")
            nc.sync.dma_start(xaT_t[:], aps["xaT"][:, sl])
            cnt_t = dense.tile([1, NT], f32, tag="cnt")
            nc.sync.dma_start(cnt_t[:], aps["counts"][:, sl])

            # d = x_dist @ Wd
            ps_d = psc.tile([H, NT], f32, tag="ps")
            mm(ps_d, wt["Wd"], xdT_t)
            d_sb = dense.tile([H, NT], f32, tag="d")
            nc.scalar.copy(d_sb[:], ps_d[:])

            # u = (S@Ws + c*bs) * d
            ps_u = psc.tile([H, NT], f32, tag="ps")
            nc.tensor.matmul(ps_u[:], lhsT=wt["Ws"][:], rhs=ST[:, sl],
                             start=True, stop=False, skip_group_check=True)
            mm(ps_u, wt["bs_row"], cnt_t, start=False, stop=True)
            u_sb = dense.tile([H, NT], f32, tag="u")
            nc.vector.tensor_tensor(out=u_sb[:], in0=ps_u[:], in1=d_sb[:], op=mult)

            # x0 = agg + msg@Wt + bt    (accumulated in one PSUM tile)
            ps_x0 = pacc.tile([H, NT], f32, tag="pacc")
            mm(ps_x0, wt["Wt"], msgT_t, start=True, stop=False)
            mm(ps_x0, wt["bt_row"], ones_row[:, sl], start=False, stop=False)
            for b in range(BD):
                bsl = slice(b * 128, (b + 1) * 128)
                ps_a = psc.tile([H, NT], f32, tag="ps")
                mm(ps_a, wt["WaRep"][:, bsl], xaT_t)
                z_sb = dense.tile([H, NT], f32, tag="z")
                nc.vector.tensor_tensor(out=z_sb[:], in0=u_sb[:], in1=ps_a[:],
                                        op=mult)
                mm(ps_x0, wt["WbT"][:, bsl], z_sb, start=False,
                   stop=(b == BD - 1))
            x0_sb = dense.tile([H, NT], f32, tag="x0")
            nc.scalar.copy(x0_sb[:], ps_x0[:])

            # residual block (H)
            ps_h = psc.tile([H, NT], f32, tag="ps")
            mm(ps_h, wt["rb_w1"], x0_sb)
            h1_sb = dense.tile([H, NT], f32, tag="h1")
            nc.scalar.activation(h1_sb[:], ps_h[:], Silu, bias=wt["rb_b1"][:])
            ps_h2 = psc.tile([H, NT], f32, tag="ps")
            mm(ps_h2, wt["rb_w2"], h1_sb)
            h2_sb = dense.tile([H, NT], f32, tag="h2")
            nc.scalar.activation(h2_sb[:], ps_h2[:], Silu, bias=wt["rb_b2"][:])

            # skip: y = silu((x0+h2)@Wskip + bskip) + msg
            ps_y = pacc.tile([H, NT], f32, tag="pacc")
            mm(ps_y, wt["Wskip"], x0_sb, start=True, stop=False)
            mm(ps_y, wt["Wskip"], h2_sb, start=False, stop=True)
            ys_sb = dense.tile([M, NT], f32, tag="ys")
            nc.scalar.activation(ys_sb[:], ps_y[:], Silu, bias=wt["bskip"][:])
            y_sb = dense.tile([M, NT], f32, tag="y")
            nc.vector.tensor_add(out=y_sb[:], in0=ys_sb[:], in1=msgT_t[:])

            # residual after 1
            ps_h = psc.tile([M, NT], f32, tag="ps")
            mm(ps_h, wt["ra1_w1"], y_sb)
            h1p = dense.tile([M, NT], f32, tag="h1")
            nc.scalar.activation(h1p[:], ps_h[:], Silu, bias=wt["ra1_b1"][:])
            ps_h2 = psc.tile([M, NT], f32, tag="ps")
            mm(ps_h2, wt["ra1_w2"], h1p)
            h2p = dense.tile([M, NT], f32, tag="h2")
            nc.scalar.activation(h2p[:], ps_h2[:], Silu, bias=wt["ra1_b2"][:])
            x2_sb = dense.tile([M, NT], f32, tag="x2")
            nc.vector.tensor_add(out=x2_sb[:], in0=y_sb[:], in1=h2p[:])

            # residual after 2
            ps_h = psc.tile([M, NT], f32, tag="ps")
            mm(ps_h, wt["ra2_w1"], x2_sb)
            h1q = dense.tile([M, NT], f32, tag="h1")
            nc.scalar.activation(h1q[:], ps_h[:], Silu, bias=wt["ra2_b1"][:])
            ps_h2 = psc.tile([M, NT], f32, tag="ps")
            mm(ps_h2, wt["ra2_w2"], h1q)
            h2q = dense.tile([M, NT], f32, tag="h2")
            nc.scalar.activation(h2q[:], ps_h2[:], Silu, bias=wt["ra2_b2"][:])
            o_sb = dense.tile([M, NT], f32, tag="o")
            nc.vector.tensor_add(out=o_sb[:], in0=x2_sb[:], in1=h2q[:])

            nc.sync.dma_start(aps["outT"][:, sl], o_sb[:])


def kernel(**inputs):
    _tlog("kernel() start")
    inputs = {k: np.asarray(v) for k, v in inputs.items()}
    x_dist = inputs["x_dist"].astype(np.float32)
    x_angle = inputs["x_angle"].astype(np.float32)
    msg = inputs["msg"].astype(np.float32)
    angle_index = inputs["angle_index"]

    in_maps, perms, ncols = _preprocess(x_dist, x_angle, msg, angle_index)
    _tlog("preprocess done")
    wmap = _weights_maps(**{k: inputs[k] for k in (
        "Wd", "Wa", "Ws", "bs", "Wt", "bt", "Wb",
        "rb_w1", "rb_b1", "rb_w2", "rb_b2", "Wskip", "bskip",
        "ra1_w1", "ra1_b1", "ra1_w2", "ra1_b2",
        "ra2_w1", "ra2_b1", "ra2_w2", "ra2_b2")})
    for im in in_maps:
        im.update(wmap)
    _tlog("weights done")

    import concourse.bass as bass
    import concourse.tile as tile
    from concourse import bacc, mybir
    from concourse import bass_utils
    _tlog("imports done")

    nc = bacc.Bacc("TRN2", target_bir_lowering=False, debug=False,
                   enable_asserts=False, num_devices=P)
    aps = {}
    for name, arr in in_maps[0].items():
        aps[name] = nc.dram_tensor(
            name, arr.shape, mybir.dt.from_np(arr.dtype),
            kind="ExternalInput").ap()
    aps["outT"] = nc.dram_tensor(
        "outT", (M, NSP), mybir.dt.float32, kind="ExternalOutput").ap()

    with tile.TileContext(nc) as tc:
        _build(nc, tc, ncols, aps)
    _tlog("IR build + tile schedule done")
    nc.compile()
    _tlog("nc.compile done")

    res = bass_utils.run_bass_kernel_spmd(
        nc, in_maps, core_ids=list(range(P)),
        trace=bool(int(os.environ.get("KTRACE", "0"))),
        stitch_traces=bool(int(os.environ.get("KTRACE", "0"))),
        trace_cores=list(range(P)) if int(os.environ.get("KTRACE", "0")) else None,
    )
    _tlog("run_bass_kernel_spmd done")
    kernel.last_results = res

    out = np.empty((E, M), np.float32)
    for p in range(P):
        outT = res.results[p]["outT"]  # [M, NSP]
        out[p * ES + perms[p]] = outT[:, :ES].T
    _tlog("output gather done")
    return out.astype(inputs["msg"].dtype if inputs["msg"].dtype == np.float32
                      else np.float32)



# revision 7
# speedup vs baseline: 5.1375x; 3.9495x over previous
"""Trainium2 Bass kernel for nn_InteractionBlock (gnn_message_passing).

Algebra: per angle alpha with (s, t) = (src, tgt):
    sm[alpha] = (msg[s] @ Ws + bs) * d[t]
    out[alpha] = sum_b a[t, b] * (Wb[:, b, :] @ sm[alpha])
    agg[t] = sum_{alpha: tgt=t} out[alpha]
Everything except msg[s] depends only on t, so with
    S[t] = sum_{alpha: tgt=t} msg[s(alpha)]   and  c[t] = |{alpha: tgt=t}|
    agg[t] = sum_b a[t,b] * (Wb[:,b,:] @ ((S[t] @ Ws + c[t]*bs) * d[t]))
The A=600K gather+einsum+scatter collapses to an E-sized dense pipeline after
a segment-sum of raw msg rows.

Distribution: edges (targets) are sharded across 8 cores. The host↔device
link is slow (~10-50 MB/s), so the big msg table is NOT replicated on the
wire: each core ships only its 12.5K-row slice in bf16 and the full table is
rebuilt on-device with an HBM AllGather over NeuronLink. Activations cross
the wire in bf16; the output returns in bf16 (rel-err budget 2e-2 is plenty).
Host does index-only preprocessing (permutations / slot tables / layout
transposes / dtype casts).
"""

import os
import sys
import time

import numpy as np

sys.path.insert(0, "/opt/trn_rl_repo")

_T0 = time.perf_counter()


def _tlog(msg):
    print(f"[ktime +{time.perf_counter() - _T0:7.2f}s] {msg}", file=sys.stderr)


E = 100000
A = 600000
NR = 6
NS = 7
H = 128
BD = 8
M = 128
P = 8            # cores
ES = E // P      # 12500 edges per core
ESP = ES + 1     # slice rows shipped per core (incl one zero row)
RT = ESP * P     # AllGather row count
ZROW = ES        # zero row index within block 0 of the AllGather buffer
NT = 512         # dense-phase column tile
NCH = 100        # chunks of 128 targets per core (padded)
NSP = NCH * 128  # 12800 padded targets per core
NTILES = NSP // NT  # 25


def _remap(s):
    """msg row id -> row in the AllGather buffer (blocks of ESP rows/core)."""
    return (s // ES) * ESP + (s % ES)


def _preprocess(x_dist, x_angle, msg, angle_index):
    """Index-only + layout host prep. Returns per-core input maps and meta."""
    import ml_dtypes
    bf16 = ml_dtypes.bfloat16

    src = angle_index[0].astype(np.int64)
    tgt = angle_index[1].astype(np.int64)

    cnt = np.bincount(tgt, minlength=E).astype(np.int64)
    order = np.argsort(tgt, kind="stable")
    srcs_by_tgt = src[order].astype(np.int64)
    starts = np.zeros(E + 1, np.int64)
    np.cumsum(cnt, out=starts[1:])

    perms = []
    core_cols = []
    for p in range(P):
        lo = p * ES
        cp = cnt[lo:lo + ES]
        perm = np.argsort(-cp, kind="stable")
        perms.append(perm)
        cps_pad = np.zeros(NSP, np.int64)
        cps_pad[:ES] = cp[perm]
        core_cols.append(cps_pad.reshape(NCH, 128).max(axis=1))
    ncols = np.maximum.reduce(core_cols)
    ncols = np.maximum(ncols, 1)
    tot_cols = int(ncols.sum())
    maxc = int(ncols.max())

    xa_flat = x_angle.reshape(E, NS * NR)
    in_maps = []
    for p in range(P):
        lo = p * ES
        perm = perms[p]
        gperm = perm + lo
        cps = cnt[gperm]

        # slot table: column r of chunk c, partition k holds the j-th source
        # of (new-label) target 128c+k, remapped to AllGather rows; ZROW pads.
        slot = np.full((NSP, maxc), ZROW, np.int64)
        reps = cps
        ii = np.repeat(np.arange(ES), reps)
        jj = (np.arange(len(ii)) -
              np.repeat(np.concatenate(([0], np.cumsum(reps)[:-1])), reps))
        gt = np.repeat(gperm, reps)
        slot[ii, jj] = _remap(srcs_by_tgt[starts[gt] + jj])

        cols = np.concatenate(
            [slot[c * 128:(c + 1) * 128, :ncols[c]] for c in range(NCH)],
            axis=1,
        ).astype(np.int32)  # [128, tot_cols]

        selfs = np.full((NSP,), ZROW, np.int64)
        selfs[:ES] = _remap(gperm)
        selfs = selfs.reshape(NCH, 128).T.astype(np.int32)  # [128, NCH]

        mslice = np.zeros((ESP, M), bf16)
        mslice[:ES] = msg[lo:lo + ES].astype(bf16)

        xaT = np.zeros((NS * NR, NSP), bf16)
        xaT[:, :ES] = xa_flat[lo:lo + ES][perm].T.astype(bf16)
        xdT = np.zeros((NR, NSP), bf16)
        xdT[:, :ES] = x_dist[lo:lo + ES][perm].T.astype(bf16)
        counts_row = np.zeros((1, NSP), np.float32)
        counts_row[0, :ES] = cps.astype(np.float32)

        in_maps.append({
            "mslice": mslice,
            "slots": np.ascontiguousarray(cols),
            "selfs": np.ascontiguousarray(selfs),
            "xaT": xaT,
            "xdT": xdT,
            "counts": counts_row,
        })
    return in_maps, perms, ncols


def _weights_maps(Wd, Wa, Ws, bs, Wt, bt, Wb, rb_w1, rb_b1, rb_w2, rb_b2,
                  Wskip, bskip, ra1_w1, ra1_b1, ra1_w2, ra1_b2,
                  ra2_w1, ra2_b1, ra2_w2, ra2_b2):
    import ml_dtypes
    bf16 = ml_dtypes.bfloat16
    f32 = np.float32
    WaRep = np.empty((NS * NR, BD * 128), bf16)
    for b in range(BD):
        WaRep[:, b * 128:(b + 1) * 128] = Wa[:, b:b + 1].astype(bf16)
    WbT = np.empty((H, BD * H), f32)
    for b in range(BD):
        WbT[:, b * 128:(b + 1) * 128] = Wb[:, b, :].T
    return {
        "Wd": Wd.astype(bf16), "WaRep": WaRep,
        "Ws": Ws.astype(f32), "bs_row": bs.reshape(1, H).astype(f32),
        "Wt": Wt.astype(f32), "bt_row": bt.reshape(1, H).astype(f32),
        "WbT": WbT,
        "rb_w1": rb_w1.astype(f32), "rb_b1": rb_b1.reshape(H, 1).astype(f32),
        "rb_w2": rb_w2.astype(f32), "rb_b2": rb_b2.reshape(H, 1).astype(f32),
        "Wskip": Wskip.astype(f32), "bskip": bskip.reshape(M, 1).astype(f32),
        "ra1_w1": ra1_w1.astype(f32), "ra1_b1": ra1_b1.reshape(M, 1).astype(f32),
        "ra1_w2": ra1_w2.astype(f32), "ra1_b2": ra1_b2.reshape(M, 1).astype(f32),
        "ra2_w1": ra2_w1.astype(f32), "ra2_b1": ra2_b1.reshape(M, 1).astype(f32),
        "ra2_w2": ra2_w2.astype(f32), "ra2_b2": ra2_b2.reshape(M, 1).astype(f32),
    }


def _build(nc, tc, ncols, aps):
    from contextlib import ExitStack

    import concourse.bass as bass
    from concourse import mybir
    from concourse.bass import IndirectOffsetOnAxis
    from concourse.masks import make_identity

    f32 = mybir.dt.float32
    bf16 = mybir.dt.bfloat16
    Silu = mybir.ActivationFunctionType.Silu
    mult = mybir.AluOpType.mult
    tot_cols = int(ncols.sum())

    with ExitStack() as ctx:
        wpool = ctx.enter_context(tc.tile_pool(name="w", bufs=1))
        stpool = ctx.enter_context(tc.tile_pool(name="st", bufs=1))
        dpool = ctx.enter_context(tc.tile_pool(name="dram", bufs=1, space="DRAM"))

        # ship msg slice to an internal DRAM bounce, AllGather the full table
        mbounce = dpool.tile([ESP, M], bf16)
        nc.gpsimd.dma_start(mbounce[:], aps["mslice"][:])
        ag = dpool.tile([RT, M], bf16)
        nc.gpsimd.collective_compute(
            "AllGather", mybir.AluOpType.bypass,
            replica_groups=[list(range(P))],
            ins=[mbounce.opt()], outs=[ag.opt()],
        )

        # resident weights
        wt = {}
        for name, shape, dt in [
            ("Wd", (NR, H), bf16), ("WaRep", (NS * NR, BD * 128), bf16),
            ("Ws", (M, H), f32), ("bs_row", (1, H), f32),
            ("Wt", (M, H), f32), ("bt_row", (1, H), f32),
            ("WbT", (H, BD * H), f32),
            ("rb_w1", (H, H), f32), ("rb_b1", (H, 1), f32),
            ("rb_w2", (H, H), f32), ("rb_b2", (H, 1), f32),
            ("Wskip", (H, M), f32), ("bskip", (M, 1), f32),
            ("ra1_w1", (M, M), f32), ("ra1_b1", (M, 1), f32),
            ("ra1_w2", (M, M), f32), ("ra1_b2", (M, 1), f32),
            ("ra2_w1", (M, M), f32), ("ra2_b1", (M, 1), f32),
            ("ra2_w2", (M, M), f32), ("ra2_b2", (M, 1), f32),
        ]:
            t = wpool.tile(list(shape), dt, tag=name)
            nc.sync.dma_start(t[:], aps[name][:])
            wt[name] = t

        identb = wpool.tile([128, 128], bf16, tag="identb")
        make_identity(nc, identb[:])
        ones_row = wpool.tile([1, NT], f32, tag="ones")
        nc.gpsimd.memset(ones_row[:], 1.0)

        slots_sb = wpool.tile([128, tot_cols], mybir.dt.int32, tag="slots")
        nc.sync.dma_start(slots_sb[:], aps["slots"][:])
        selfs_sb = wpool.tile([128, NCH], mybir.dt.int32, tag="selfs")
        nc.sync.dma_start(selfs_sb[:], aps["selfs"][:])

        ST = stpool.tile([M, NSP], f32, tag="ST")  # segment sums, feature-major
        MT = stpool.tile([M, NSP], f32, tag="MT")  # own msg rows, feature-major

        # ---- phase 1: S^T and msg^T chunks via gathers + transpose-matmuls --
        with tc.tile_pool(name="g", bufs=12) as gpool, \
             tc.tile_pool(name="pt", bufs=4, space="PSUM") as ptpool, \
             tc.tile_pool(name="pv", bufs=2, space="PSUM") as pvpool:
            col = 0
            for c in range(NCH):
                nj = int(ncols[c])
                csl = slice(c * 128, (c + 1) * 128)
                ps = ptpool.tile([128, 128], f32, tag="pt")
                for j in range(nj):
                    g = gpool.tile([128, M], bf16, tag="g")
                    nc.gpsimd.indirect_dma_start(
                        out=g[:], out_offset=None,
                        in_=ag[:],
                        in_offset=IndirectOffsetOnAxis(
                            ap=slots_sb[:, col + j:col + j + 1], axis=0),
                    )
                    nc.tensor.matmul(
                        ps[:], lhsT=g[:], rhs=identb[:],
                        start=(j == 0), stop=(j == nj - 1),
                        skip_group_check=True,
                    )
                nc.scalar.copy(ST[:, csl], ps[:])
                col += nj

                gs = gpool.tile([128, M], bf16, tag="gs")
                nc.gpsimd.indirect_dma_start(
                    out=gs[:], out_offset=None,
                    in_=ag[:],
                    in_offset=IndirectOffsetOnAxis(
                        ap=selfs_sb[:, c:c + 1], axis=0),
                )
                ps2 = pvpool.tile([128, 128], f32, tag="pv")
                nc.tensor.matmul(ps2[:], lhsT=gs[:], rhs=identb[:],
                                 start=True, stop=True, skip_group_check=True)
                nc.scalar.copy(MT[:, csl], ps2[:])

        # ---- phase 2: dense per-edge pipeline, feature-major ----------------
        dense = ctx.enter_context(tc.tile_pool(name="dn", bufs=2))
        pacc = ctx.enter_context(tc.tile_pool(name="pacc", bufs=2, space="PSUM"))
        psc = ctx.enter_context(tc.tile_pool(name="psc", bufs=4, space="PSUM"))

        def mm(out, lhsT, rhs, start=True, stop=True):
            nc.tensor.matmul(out[:], lhsT=lhsT[:], rhs=rhs[:], start=start,
                             stop=stop, skip_group_check=True)

        for t in range(NTILES):
            sl = slice(t * NT, (t + 1) * NT)

            xdT_t = dense.tile([NR, NT], bf16, tag="xdT")
            nc.sync.dma_start(xdT_t[:], aps["xdT"][:, sl])
            xaT_t = dense.tile([NS * NR, NT], bf16, tag="xaT")
            nc.sync.dma_start(xaT_t[:], aps["xaT"][:, sl])
            cnt_t = dense.tile([1, NT], f32, tag="cnt")
            nc.sync.dma_start(cnt_t[:], aps["counts"][:, sl])

            # d = x_dist @ Wd
            ps_d = psc.tile([H, NT], f32, tag="ps")
            mm(ps_d, wt["Wd"], xdT_t)
            d_sb = dense.tile([H, NT], f32, tag="d")
            nc.scalar.copy(d_sb[:], ps_d[:])

            # u = (S@Ws + c*bs) * d
            ps_u = psc.tile([H, NT], f32, tag="ps")
            nc.tensor.matmul(ps_u[:], lhsT=wt["Ws"][:], rhs=ST[:, sl],
                             start=True, stop=False, skip_group_check=True)
            mm(ps_u, wt["bs_row"], cnt_t, start=False, stop=True)
            u_sb = dense.tile([H, NT], f32, tag="u")
            nc.vector.tensor_tensor(out=u_sb[:], in0=ps_u[:], in1=d_sb[:], op=mult)

            # x0 = agg + msg@Wt + bt    (accumulated in one PSUM tile)
            ps_x0 = pacc.tile([H, NT], f32, tag="pacc")
            nc.tensor.matmul(ps_x0[:], lhsT=wt["Wt"][:], rhs=MT[:, sl],
                             start=True, stop=False, skip_group_check=True)
            nc.tensor.matmul(ps_x0[:], lhsT=wt["bt_row"][:], rhs=ones_row[:],
                             start=False, stop=False, skip_group_check=True)
            for b in range(BD):
                bsl = slice(b * 128, (b + 1) * 128)
                ps_a = psc.tile([H, NT], f32, tag="ps")
                mm(ps_a, wt["WaRep"][:, bsl], xaT_t)
                z_sb = dense.tile([H, NT], f32, tag="z")
                nc.vector.tensor_tensor(out=z_sb[:], in0=u_sb[:], in1=ps_a[:],
                                        op=mult)
                mm(ps_x0, wt["WbT"][:, bsl], z_sb, start=False,
                   stop=(b == BD - 1))
            x0_sb = dense.tile([H, NT], f32, tag="x0")
            nc.scalar.copy(x0_sb[:], ps_x0[:])

            # residual block (H)
            ps_h = psc.tile([H, NT], f32, tag="ps")
            mm(ps_h, wt["rb_w1"], x0_sb)
            h1_sb = dense.tile([H, NT], f32, tag="h1")
            nc.scalar.activation(h1_sb[:], ps_h[:], Silu, bias=wt["rb_b1"][:])
            ps_h2 = psc.tile([H, NT], f32, tag="ps")
            mm(ps_h2, wt["rb_w2"], h1_sb)
            h2_sb = dense.tile([H, NT], f32, tag="h2")
            nc.scalar.activation(h2_sb[:], ps_h2[:], Silu, bias=wt["rb_b2"][:])

            # skip: y = silu((x0+h2)@Wskip + bskip) + msg
            ps_y = pacc.tile([H, NT], f32, tag="pacc")
            mm(ps_y, wt["Wskip"], x0_sb, start=True, stop=False)
            mm(ps_y, wt["Wskip"], h2_sb, start=False, stop=True)
            ys_sb = dense.tile([M, NT], f32, tag="ys")
            nc.scalar.activation(ys_sb[:], ps_y[:], Silu, bias=wt["bskip"][:])
            y_sb = dense.tile([M, NT], f32, tag="y")
            nc.vector.tensor_add(out=y_sb[:], in0=ys_sb[:], in1=MT[:, sl])

            # residual after 1
            ps_h = psc.tile([M, NT], f32, tag="ps")
            mm(ps_h, wt["ra1_w1"], y_sb)
            h1p = dense.tile([M, NT], f32, tag="h1")
            nc.scalar.activation(h1p[:], ps_h[:], Silu, bias=wt["ra1_b1"][:])
            ps_h2 = psc.tile([M, NT], f32, tag="ps")
            mm(ps_h2, wt["ra1_w2"], h1p)
            h2p = dense.tile([M, NT], f32, tag="h2")
            nc.scalar.activation(h2p[:], ps_h2[:], Silu, bias=wt["ra1_b2"][:])
            x2_sb = dense.tile([M, NT], f32, tag="x2")
            nc.vector.tensor_add(out=x2_sb[:], in0=y_sb[:], in1=h2p[:])

            # residual after 2
            ps_h = psc.tile([M, NT], f32, tag="ps")
            mm(ps_h, wt["ra2_w1"], x2_sb)
            h1q = dense.tile([M, NT], f32, tag="h1")
            nc.scalar.activation(h1q[:], ps_h[:], Silu, bias=wt["ra2_b1"][:])
            ps_h2 = psc.tile([M, NT], f32, tag="ps")
            mm(ps_h2, wt["ra2_w2"], h1q)
            h2q = dense.tile([M, NT], f32, tag="h2")
            nc.scalar.activation(h2q[:], ps_h2[:], Silu, bias=wt["ra2_b2"][:])
            o_sb = dense.tile([M, NT], f32, tag="o")
            nc.vector.tensor_add(out=o_sb[:], in0=x2_sb[:], in1=h2q[:])
            ob = dense.tile([M, NT], bf16, tag="ob")
            nc.scalar.copy(ob[:], o_sb[:])

            nc.sync.dma_start(aps["outT"][:, sl], ob[:])


def kernel(**inputs):
    _tlog("kernel() start")
    inputs = {k: np.asarray(v) for k, v in inputs.items()}
    x_dist = inputs["x_dist"].astype(np.float32)
    x_angle = inputs["x_angle"].astype(np.float32)
    msg = inputs["msg"].astype(np.float32)
    angle_index = inputs["angle_index"]

    in_maps, perms, ncols = _preprocess(x_dist, x_angle, msg, angle_index)
    wmap = _weights_maps(**{k: inputs[k].astype(np.float32) for k in (
        "Wd", "Wa", "Ws", "bs", "Wt", "bt", "Wb",
        "rb_w1", "rb_b1", "rb_w2", "rb_b2", "Wskip", "bskip",
        "ra1_w1", "ra1_b1", "ra1_w2", "ra1_b2",
        "ra2_w1", "ra2_b1", "ra2_w2", "ra2_b2")})
    for im in in_maps:
        im.update(wmap)
    _tlog("preprocess done")

    import concourse.bass as bass
    import concourse.tile as tile
    from concourse import bacc, mybir
    from concourse import bass_utils
    _tlog("imports done")

    nc = bacc.Bacc("TRN2", target_bir_lowering=False, debug=False,
                   enable_asserts=False, num_devices=P)
    aps = {}
    for name, arr in in_maps[0].items():
        aps[name] = nc.dram_tensor(
            name, arr.shape, mybir.dt.from_np(arr.dtype),
            kind="ExternalInput").ap()
    aps["outT"] = nc.dram_tensor(
        "outT", (M, NSP), mybir.dt.bfloat16, kind="ExternalOutput").ap()

    with tile.TileContext(nc) as tc:
        _build(nc, tc, ncols, aps)
    _tlog("IR build + tile schedule done")
    nc.compile()
    _tlog("nc.compile done")

    res = bass_utils.run_bass_kernel_spmd(
        nc, in_maps, core_ids=list(range(P)),
        trace=bool(int(os.environ.get("KTRACE", "0"))),
        stitch_traces=bool(int(os.environ.get("KTRACE", "0"))),
        trace_cores=list(range(P)) if int(os.environ.get("KTRACE", "0")) else None,
    )
    _tlog("run_bass_kernel_spmd done")
    kernel.last_results = res

    out = np.empty((E, M), np.float32)
    for p in range(P):
        outT = res.results[p]["outT"]  # [M, NSP] bf16
        out[p * ES + perms[p]] = outT[:, :ES].astype(np.float32).T
    _tlog("output gather done")
    return out
